# revision 4
# baseline (speedup 1.0000x reference)
"""Trainium2 Bass kernel for nn_DivEncoder (grouped MLP + ELU + L2 norm).

Math (per batch row n):
  xg = x.reshape(D, V); zeta = einsum('duv,dv->du', W1, xg) + b1
  y_d = b2_d + sum_u W2[d,u] * elu(zeta[d,u]);  out = y / max(||y||, eps)

Decomposition on device (m = min(zeta,0), e = exp(m)):
  elu(zeta) = zeta - m + e - 1
  y = c0 + sum_v wlin[d,v] x[d,v] + sum_u W2 e - sum_u W2 m
  c0 = b2 + sum_u W2 b1 - sum_u W2 ;  wlin = sum_u W2[d,u] W1[d,u,:]

Sharding: batch rows across 8 cores (512 rows each); weights replicated.

Per-core dataflow, 64 chunks of 128 features (8 groups of 16 v's):
  - One DMA loads x[:,128c:128c+128] as [128 batch, 512] (4 batch tiles).
  - GPSIMD splits fp32 -> bf16 hi/lo, interleaved [hi32|lo32|hi32|lo32]
    per half-chunk; one batched xbar DMA transpose per half gives
    feature-major [128, 512] bf16 (partitions = hi/lo feature strips).
  - L1: 2 bf16 matmuls per 2-group block, K=64 = [hi32;lo32]:
    (W1h|W1l-stacked x 2 cross variants) accumulate exact fp32 z in PSUM.
  - m = min(z + b1, 0) on DVE (PSUM->SBUF, fp16); e = exp(m) on ACT.
  - L2: fp16 matmuls (+W2 e, -W2 m; M=32 zero-padded lhsT) plus bf16 wlin
    matmuls (M=64 per half) accumulate 16 chunks into one PSUM bank at
    slot partitions 32k + 2*(c%16) + i.
  - Evac bank (+c0) once per 16 chunks; PE transpose against a
    permutation matrix yields batch-major d-ordered tiles; row norm
    (sqrt + exact reciprocal + one Newton step); contiguous DMA out.
"""
import sys
sys.path.insert(0, "/opt/trn_rl_repo")

import numpy as np
import ml_dtypes

import concourse.bass as bass
import concourse.bacc as bacc
import concourse.mybir as mybir
import concourse.tile as tile
from concourse import bass_utils

F32 = mybir.dt.float32
F16 = mybir.dt.float16
BF16 = mybir.dt.bfloat16
AL = mybir.AluOpType
AF = mybir.ActivationFunctionType

N, H, D, U, V = 4096, 8192, 512, 64, 16
NCORE = 8
R = N // NCORE          # 512 batch rows per core
CH = H // 128           # 64 chunks
BG = 4                  # bank groups (16 chunks each)
EPS = 1e-12

_cache = {}
ABLATE_EM = False
ABLATE_ME = False


def _act_chain(c):
    """Chunks whose m-extraction runs on ACT (as q=relu(-zeta)) not DVE."""
    return c % 6 == 5


def _build(loop_reps=1):
    nc = bacc.Bacc("TRN2", target_bir_lowering=False, debug=False,
                   enable_asserts=False, num_devices=NCORE)
    ap = {}
    ap["x"] = nc.dram_tensor("x", [R, H], F32, kind="ExternalInput").ap()
    ap["w1f"] = nc.dram_tensor("w1f", [CH, 128, 128], F16, kind="ExternalInput").ap()
    ap["wlh"] = nc.dram_tensor("wlh", [CH, 128, 128], F16, kind="ExternalInput").ap()
    ap["wlr"] = nc.dram_tensor("wlr", [CH, 128, 128], F16, kind="ExternalInput").ap()
    ap["w2e"] = nc.dram_tensor("w2e", [CH, 128, 128], F16, kind="ExternalInput").ap()
    ap["w2m"] = nc.dram_tensor("w2m", [CH, 128, 128], F16, kind="ExternalInput").ap()
    ap["b1c"] = nc.dram_tensor("b1c", [CH, 128, 4], F32, kind="ExternalInput").ap()
    ap["c0s"] = nc.dram_tensor("c0s", [BG, 128, 1], F32, kind="ExternalInput").ap()
    ap["ident"] = nc.dram_tensor("ident", [128, 128], F32, kind="ExternalInput").ap()
    y_out = nc.dram_tensor("y", [R, D], F32, kind="ExternalOutput").ap()

    with tile.TileContext(nc) as tc:
        _emit(nc, tc, ap, y_out, loop_reps)
    nc.compile()
    return nc


def _emit(nc, tc, ap, y_out, loop_reps=1):
    with (
        tc.tile_pool(name="wres", bufs=1) as wres,
        tc.tile_pool(name="xin", bufs=4) as xin,
        tc.tile_pool(name="xsp", bufs=4) as xsp,
        tc.tile_pool(name="xtr", bufs=4) as xtr,
        tc.tile_pool(name="me", bufs=4) as mepool,
        tc.tile_pool(name="yfm", bufs=1) as yfm,
        tc.tile_pool(name="zps", bufs=3, space="PSUM") as zps,
        tc.tile_pool(name="yps", bufs=2, space="PSUM") as yps,
        tc.tile_pool(name="sml", bufs=1) as sml,
    ):
        # ---- resident weights
        t_w1a, t_wl1, t_wl2, t_w2e, t_w2m, t_b1 = [], [], [], [], [], []
        for c in range(CH):
            w1a = wres.tile([128, 128], F16, tag=f"w1a{c}", name=f"w1a{c}")
            nc.sync.dma_start(w1a[:], ap["w1f"][c])
            t_w1a.append(w1a)
            wl1 = wres.tile([128, 128], F16, tag=f"wl1{c}")
            nc.sync.dma_start(wl1[:], ap["wlh"][c])
            t_wl1.append(wl1)
            wl2 = wres.tile([128, 128], F16, tag=f"wl2{c}")
            nc.sync.dma_start(wl2[:], ap["wlr"][c])
            t_wl2.append(wl2)
            w2e = wres.tile([128, 128], F16, tag=f"w2e{c}")
            nc.sync.dma_start(w2e[:], ap["w2e"][c])
            t_w2e.append(w2e)
            w2m = wres.tile([128, 128], F16, tag=f"w2m{c}")
            nc.sync.dma_start(w2m[:], ap["w2m"][c])
            t_w2m.append(w2m)
            b1 = wres.tile([128, 4], F32, tag=f"b1{c}")
            nc.sync.dma_start(b1[:], ap["b1c"][c])
            t_b1.append(b1)
        t_c0 = []
        for b in range(BG):
            c0 = wres.tile([128, 1], F32, tag=f"c0{b}")
            nc.sync.dma_start(c0[:], ap["c0s"][b])
            t_c0.append(c0)
        t_id = wres.tile([128, 128], F32, tag="ident")
        nc.sync.dma_start(t_id[:], ap["ident"][:])

        x_ap = ap["x"]
        import contextlib
        loop_cm = (tc.For_i(0, loop_reps, 1, hint_engines=(mybir.EngineType.PE, mybir.EngineType.DVE))
                   if loop_reps > 1 else contextlib.nullcontext())
        with loop_cm:
            y_banks = {}
            t_yfm = [yfm.tile([128, 512], F32, tag=f"yfm{b}", name=f"yfm{b}")
                     for b in range(BG)]

            pend2 = [None, None]
            for c in range(CH + 2):
                if c < CH:
                    b = c // 16
                    cp = c % 16
                    if cp == 0:
                        y_banks[b] = yps.tile([128, 512], F32, tag="ybank",
                                              name=f"ybank{b}")
                    ybank = y_banks[b]
                    m_t = mepool.tile([128, 2048], F16, tag="m", name=f"m{c}")
                    e_t = mepool.tile([128, 2048], F16, tag="e", name=f"e{c}")

                    # --- load x chunk: one DMA, [128 batch, 4 j x 128 feats]
                    xt = xin.tile([128, 512], F32, tag="xc", name=f"xt{c}")
                    leng = nc.scalar if (c % 2 == 0) else nc.sync
                    leng.dma_start(
                        xt[:].rearrange("p (j f) -> p j f", j=4),
                        x_ap[:, 128 * c:128 * (c + 1)].rearrange(
                            "(j p) f -> p j f", p=128))

                    # --- GPSIMD cast to fp16; one batched transpose
                    xf = xsp.tile([128, 512], F16, tag="xf", name=f"xf{c}")
                    nc.gpsimd.tensor_copy(xf[:], xt[:])
                    xfT = xtr.tile([128, 512], F16, tag="xfT", name=f"xfT{c}")
                    teng = nc.sync if (c % 2 == 0) else nc.scalar
                    teng.dma_start_transpose(
                        xfT[:].rearrange("p (j n) -> p j n", j=4), xf[:])

                    # --- L1: 1 matmul per block (K=32 row strips)
                    zAB = [zps.tile([128, 1024], F32, tag="z", name=f"z{c}_{h}")
                           for h in range(2)]
                    for k in range(4):
                        zsl = zAB[k // 2][:, 512 * (k % 2):512 * (k % 2) + 512]
                        row = slice(32 * k, 32 * k + 32)
                        nc.tensor.matmul(zsl, t_w1a[c][row, :], xfT[row, :],
                                         start=True, stop=True,
                                         tile_position=(32 * k, 0),
                                         skip_group_check=True)
                    # --- wlin matmuls (M=128, zero-padded lhsT, fp16 pair)
                    nc.tensor.matmul(ybank[:, :], t_wl1[c][:, :], xfT[:, :],
                                     start=(cp == 0), stop=False, skip_group_check=True)
                    nc.tensor.matmul(ybank[:, :], t_wl2[c][:, :], xfT[:, :],
                                     start=False, stop=False, skip_group_check=True)
                    # --- m pass (DVE min-chain or ACT relu-chain)
                    for k in (range(4) if not ABLATE_ME else []):
                        zsl = zAB[k // 2][:, 512 * (k % 2):512 * (k % 2) + 512]
                        msl = m_t[:, 512 * k:512 * k + 512]
                        if _act_chain(c):
                            # q = relu(-(z + b1)); host packs b1c = -b1 here
                            nc.scalar.activation(msl, zsl, AF.Relu,
                                                 bias=t_b1[c][:, k:k + 1], scale=-1.0)
                        else:
                            nc.vector.tensor_scalar(msl, zsl, t_b1[c][:, k:k + 1],
                                                    0.0, AL.add, AL.min)
                    # --- e pass (ACT); exp(-q) for ACT-chain chunks
                    if not ABLATE_ME:
                        esc = -1.0 if _act_chain(c) else 1.0
                        nc.scalar.activation(e_t[:], m_t[:], AF.Exp, scale=esc)

                    def em_mms(c=c, m_t=m_t, e_t=e_t):
                        b = c // 16
                        ybk = y_banks[b]
                        last_chunk = (c % 16 == 15)
                        if not ABLATE_EM:
                            for k in range(4):
                                esl = e_t[:, 512 * k:512 * k + 512]
                                msl = m_t[:, 512 * k:512 * k + 512]
                                ysl = ybk[32 * k:32 * k + 32, :]
                                nc.tensor.matmul(
                                    ysl, t_w2e[c][:, 32 * k:32 * k + 32], esl,
                                    start=False, stop=False,
                                    tile_position=(0, 32 * k), skip_group_check=True)
                                nc.tensor.matmul(
                                    ysl, t_w2m[c][:, 32 * k:32 * k + 32], msl,
                                    start=False, stop=(last_chunk and k == 3),
                                    tile_position=(0, 32 * k), skip_group_check=True)
                        if last_chunk:
                            nc.vector.tensor_scalar(t_yfm[b][:], ybk[:],
                                                    t_c0[b][:, 0:1], None, AL.add)
                    next_pend = em_mms
                else:
                    next_pend = None
                old = pend2.pop(0)
                if old is not None:
                    old()
                pend2.append(next_pend)

            # ---- norm + output (batch-major via permuted PE transpose)
            for j in range(4):
                yT = xin.tile([128, 512], F32, tag="xc", name=f"yT{j}")
                for b in range(BG):
                    pT = zps.tile([128, 128], F32, tag="z", name=f"pT{j}_{b}")
                    nc.tensor.transpose(pT[:], t_yfm[b][:, 128 * j:128 * (j + 1)],
                                        t_id[:])
                    nc.vector.tensor_copy(yT[:, 128 * b:128 * (b + 1)], pT[:])
                sq = xin.tile([128, 512], F32, tag="xc", name=f"sq{j}")
                nc.scalar.activation(sq[:], yT[:], AF.Square)
                ss = sml.tile([128, 1], F32, tag=f"ss{j}")
                nc.vector.reduce_sum(ss[:], sq[:], axis=mybir.AxisListType.X)
                s = sml.tile([128, 1], F32, tag=f"s{j}")
                nc.scalar.activation(s[:], ss[:], AF.Sqrt)
                nc.vector.tensor_scalar(s[:], s[:], float(EPS), None, AL.max)
                r0 = sml.tile([128, 1], F32, tag=f"r0{j}")
                nc.vector.reciprocal(r0[:], s[:])
                t1 = sml.tile([128, 1], F32, tag=f"t1{j}")
                nc.vector.tensor_tensor(t1[:], r0[:], r0[:], AL.mult)
                nc.vector.tensor_tensor(t1[:], t1[:], ss[:], AL.mult)
                nc.vector.tensor_scalar(t1[:], t1[:], -0.5, 1.5, AL.mult, AL.add)
                r1 = sml.tile([128, 1], F32, tag=f"r1{j}")
                nc.vector.tensor_tensor(r1[:], r0[:], t1[:], AL.mult)
                nc.vector.tensor_scalar(yT[:], yT[:], r1[:], None, AL.mult)
                nc.scalar.dma_start(y_out[128 * j:128 * (j + 1), :], yT[:])


def _pack_host(W1, b1, W2, b2):
    bf = ml_dtypes.bfloat16
    W1 = W1.astype(np.float32)
    b1 = b1.astype(np.float32)
    W2 = W2.astype(np.float32)
    b2 = b2.astype(np.float32)

    wlin = np.einsum('du,duv->dv', W2.astype(np.float64),
                     W1.astype(np.float64)).astype(np.float32)
    c0 = b2 + (W2 * b1).sum(-1) - W2.sum(-1)

    W1h = W1.astype(np.float16)
    wlh = wlin.astype(np.float16)
    wll = (wlin - wlh.astype(np.float32)).astype(np.float16)
    W2f = W2.astype(np.float16)

    w1hi = np.zeros((CH, 128, 128), np.float16)
    wlhi = np.zeros((CH, 128, 128), np.float16)
    wllo = np.zeros((CH, 128, 128), np.float16)
    w2e = np.zeros((CH, 128, 128), np.float16)
    b1c = np.zeros((CH, 128, 4), np.float32)
    c0s = np.zeros((BG, 128, 1), np.float32)

    for c in range(CH):
        cp = c % 16
        bi = c // 16
        for k in range(4):
            g0 = 8 * c + 2 * k
            g1 = g0 + 1
            w1hi[c, 32 * k:32 * k + 16, 0:64] = W1h[g0].T
            w1hi[c, 32 * k + 16:32 * k + 32, 64:128] = W1h[g1].T
            scol = 32 * k + 2 * cp
            wlhi[c, 32 * k:32 * k + 16, scol] = wlh[g0]
            wlhi[c, 32 * k + 16:32 * k + 32, scol + 1] = wlh[g1]
            wllo[c, 32 * k:32 * k + 16, scol] = wll[g0]
            wllo[c, 32 * k + 16:32 * k + 32, scol + 1] = wll[g1]
            w2e[c, 0:64, scol] = W2f[g0]
            w2e[c, 64:128, scol + 1] = W2f[g1]
            b1c[c, 0:64, k] = b1[g0]
            b1c[c, 64:128, k] = b1[g1]
            c0s[bi, scol, 0] = c0[g0]
            c0s[bi, scol + 1, 0] = c0[g1]
    w2m = -w2e
    for c in range(CH):
        if c % 6 == 5:  # _act_chain
            b1c[c] = -b1c[c]
            w2m[c] = -w2m[c]
    # permutation matrix: transpose output col j (= d-local) <- slot s
    ident = np.zeros((128, 128), dtype=np.float32)
    for cp in range(16):
        for k in range(4):
            for i_ in range(2):
                jcol = 8 * cp + 2 * k + i_
                slot = 32 * k + 2 * cp + i_
                ident[slot, jcol] = 1.0
    return {"w1f": w1hi, "wlh": wlhi, "wlr": wllo,
            "w2e": w2e, "w2m": w2m, "b1c": b1c, "c0s": c0s, "ident": ident}


def kernel(x, W1, b1, W2, b2):
    x = np.ascontiguousarray(np.asarray(x, dtype=np.float32))
    packed = _pack_host(np.asarray(W1), np.asarray(b1),
                        np.asarray(W2), np.asarray(b2))
    if "nc" not in _cache:
        _cache["nc"] = _build()
    nc = _cache["nc"]
    in_maps = []
    for i in range(NCORE):
        m = dict(packed)
        m["x"] = x[i * R:(i + 1) * R]
        in_maps.append(m)
    res = bass_utils.run_bass_kernel_spmd(nc, in_maps, core_ids=list(range(NCORE)))
    out = np.concatenate([res.results[i]["y"] for i in range(NCORE)], axis=0)
    return out.astype(np.float32)

